# revision 15
# baseline (speedup 1.0000x reference)
"""Trainium2 Bass kernel for nn_JslBERT (embedding lookup + 4-layer BERT encoder).

Sharding: 8 cores = 4 batch x 2 head-groups. Core c handles batch b=c//2 and
heads [6g, 6g+6) with g=c%2. Per layer, attention-output partials are pairwise
AllReduced in bf16; LN+FFN run redundantly on both cores of a pair.

v2 design (vs baseline):
 - weights streamed in bf16 (half DMA + half SBUF), activations f32r/bf16 mixed
 - attention phase split: KV for all heads first (per s-chunk), then the
   Q/scores/softmax/ctx/out pipeline per 256-row t-chunk
 - the per-layer AllReduce is split into two 256-row chunks so it overlaps
   with the other chunk's compute and with the next layer's KV (s-chunk 0)
 - softmax normalization folded into the P-transpose as a diag(1/sum) matmul
 - PSUM-resident accumulations, fp32 accumulate everywhere
"""
import os
import numpy as np
import ml_dtypes

import concourse.bass as bass
import concourse.bacc as bacc
import concourse.tile as tile
import concourse.bass_utils as bass_utils
from concourse import mybir
from concourse.masks import make_identity

# Model dims (hardcoded per problem spec)
B, S, L, D, H, V, PMAX = 4, 512, 4, 768, 12, 32000, 512
EPS = 1e-3
NCORES = 8
HPC = H // 2          # heads per core
KH = D                # head dim (768)
HK = HPC * KH         # 4608 flattened head dims per core
SCALE = 1.0 / float(np.sqrt(D))

F32 = mybir.dt.float32
F32R = mybir.dt.float32r
BF16 = mybir.dt.bfloat16
I32 = mybir.dt.int32

TT = S // 128         # 4 t-tiles total
DC = D // 128         # 6 d-chunks
NCH = [(0, 512), (512, 256)]  # free-dim chunks for width-768 outputs
NTC = 2               # t-chunks per sequence
TPC = TT // NTC       # 128-tiles per chunk (2)
CW = S // NTC         # chunk width (256)

BF = np.dtype(ml_dtypes.bfloat16)


def build_nc(n_layers=L, flags=None):
    """Build the Bass graph. flags: dict of which optional inputs exist."""
    flags = flags or {}
    nc = bacc.Bacc("TRN2", target_bir_lowering=False, debug=False,
                   num_devices=NCORES)

    xids_d = nc.dram_tensor("xids", [3, S], I32, kind="ExternalInput").ap()
    tokw_d = nc.dram_tensor("tok_w", [V, D], BF16, kind="ExternalInput").ap()
    posw_d = nc.dram_tensor("pos_w", [PMAX, D], BF16, kind="ExternalInput").ap()
    segw_d = nc.dram_tensor("seg_w", [2, D], BF16, kind="ExternalInput").ap()
    wq_d = nc.dram_tensor("wq", [n_layers, D, HK], BF16, kind="ExternalInput").ap()
    wk_d = nc.dram_tensor("wk", [n_layers, D, HK], BF16, kind="ExternalInput").ap()
    wv_d = nc.dram_tensor("wv", [n_layers, D, HK], BF16, kind="ExternalInput").ap()
    wo_d = nc.dram_tensor("wo", [n_layers, HK, D], BF16, kind="ExternalInput").ap()
    ff_d = nc.dram_tensor("ff", [n_layers, D, D], BF16, kind="ExternalInput").ap()
    out_d = nc.dram_tensor("out", [S, D], F32, kind="ExternalOutput").ap()

    opt = {}
    if flags.get("emb_bias"):
        opt["emb_bias"] = nc.dram_tensor("emb_bias", [D], F32, kind="ExternalInput").ap()
    for nm in ("ln1", "ln2"):
        if flags.get(nm):
            opt[nm + "_g"] = nc.dram_tensor(nm + "_g", [n_layers, D], F32, kind="ExternalInput").ap()
            opt[nm + "_b"] = nc.dram_tensor(nm + "_b", [n_layers, D], F32, kind="ExternalInput").ap()
    if flags.get("mask"):
        opt["maskneg"] = nc.dram_tensor("maskneg", [S], F32, kind="ExternalInput").ap()

    with tile.TileContext(nc) as tc:
        import contextlib
        with contextlib.ExitStack() as ctx:
            _build_body(ctx, tc, n_layers, flags, xids_d, tokw_d, posw_d, segw_d,
                        wq_d, wk_d, wv_d, wo_d, ff_d, out_d, opt)
    nc.compile()
    return nc


def _build_body(ctx, tc, n_layers, flags, xids_d, tokw_d, posw_d, segw_d,
                wq_d, wk_d, wv_d, wo_d, ff_d, out_d, opt):
    nc = tc.nc

    const = ctx.enter_context(tc.tile_pool(name="const", bufs=1))
    w_pool = ctx.enter_context(tc.tile_pool(name="wp", bufs=20))
    rT_pool = ctx.enter_context(tc.tile_pool(name="rT", bufs=24))
    kt_pool = ctx.enter_context(tc.tile_pool(name="ktp", bufs=36))
    v_pool = ctx.enter_context(tc.tile_pool(name="vp", bufs=24))
    qt_pool = ctx.enter_context(tc.tile_pool(name="qtp", bufs=21))
    pe_pool = ctx.enter_context(tc.tile_pool(name="pep", bufs=4))
    pt_pool = ctx.enter_context(tc.tile_pool(name="ptp", bufs=9))
    ct_pool = ctx.enter_context(tc.tile_pool(name="ctp", bufs=6))
    xtd_pool = ctx.enter_context(tc.tile_pool(name="xtd", bufs=6))
    accb_pool = ctx.enter_context(tc.tile_pool(name="accb", bufs=8))
    sm_pool = ctx.enter_context(tc.tile_pool(name="sm", bufs=8))
    ps_mm = ctx.enter_context(tc.tile_pool(name="psmm", bufs=4, space="PSUM"))
    ps_pt = ctx.enter_context(tc.tile_pool(name="pspt", bufs=2, space="PSUM"))
    ps_tp = ctx.enter_context(tc.tile_pool(name="pstp", bufs=2, space="PSUM"))
    dram = ctx.enter_context(tc.tile_pool(name="dram", bufs=2, space="DRAM"))

    ident = const.tile([128, 128], F32)
    make_identity(nc, ident[:])
    identb = const.tile([128, 128], BF16)
    make_identity(nc, identb[:])
    eps_t = const.tile([128, 1], F32)
    nc.vector.memset(eps_t[:], EPS)

    def mm_tile():
        return ps_mm.tile([128, 512], F32, tag="mm", name="mmps")

    # ---- embeddings ----------------------------------------------------
    idx = const.tile([128, 3, TT], I32)
    nc.sync.dma_start(idx[:], xids_d.rearrange("k (j p) -> p k j", p=128))

    emb_bias_ap = None
    if "emb_bias" in opt:
        eb = const.tile([128, DC], F32)
        nc.sync.dma_start(eb[:], opt["emb_bias"].rearrange("(c p) -> p c", p=128))
        emb_bias_ap = [eb[:, c:c + 1] for c in range(DC)]

    x_tiles = []
    for tm in range(TT):
        xt = accb_pool.tile([128, D], BF16, tag="accb")
        tmp = accb_pool.tile([128, D], BF16, tag="accb")
        tmp2 = accb_pool.tile([128, D], BF16, tag="accb")
        nc.gpsimd.indirect_dma_start(
            out=xt[:], out_offset=None, in_=tokw_d[:],
            in_offset=bass.IndirectOffsetOnAxis(ap=idx[:, 0, tm:tm + 1], axis=0))
        nc.gpsimd.indirect_dma_start(
            out=tmp[:], out_offset=None, in_=posw_d[:],
            in_offset=bass.IndirectOffsetOnAxis(ap=idx[:, 1, tm:tm + 1], axis=0))
        nc.vector.tensor_add(xt[:], xt[:], tmp[:])
        nc.gpsimd.indirect_dma_start(
            out=tmp2[:], out_offset=None, in_=segw_d[:],
            in_offset=bass.IndirectOffsetOnAxis(ap=idx[:, 2, tm:tm + 1], axis=0))
        nc.vector.tensor_add(xt[:], xt[:], tmp2[:])
        x_tiles.append(xt)

    # resTc[tcix][dc]: [128 d, 256 t] f32r per t-chunk
    resTc = [[None] * DC for _ in range(NTC)]
    for tcix in range(NTC):
        for dc in range(DC):
            pp = ps_tp.tile([128, CW], BF16, tag="tp", name="tpps")
            for tl in range(TPC):
                nc.tensor.transpose(pp[:, tl * 128:(tl + 1) * 128],
                                    x_tiles[tcix * TPC + tl][:, dc * 128:(dc + 1) * 128],
                                    identb[:])
            rt = rT_pool.tile([128, CW], BF16, tag="rT")
            if emb_bias_ap is not None:
                nc.vector.tensor_scalar_add(rt[:], pp[:], emb_bias_ap[dc])
            else:
                nc.vector.tensor_copy(out=rt[:], in_=pp[:])
            resTc[tcix][dc] = rt

    mask_ap = None
    if "maskneg" in opt:
        mk = const.tile([128, S], F32)
        nc.sync.dma_start(mk[:], opt["maskneg"].partition_broadcast(128))
        mask_ap = mk

    # ---- per-layer helpers --------------------------------------------
    QT_AHEAD = 6  # heads whose chunk-0 QT is prefetched at the end of the previous layer

    def load_ln_gb(li, nm):
        if nm + "_g" not in opt:
            return None
        gb = const.tile([128, 2, D], F32, tag=f"lngb{nm}{li}")
        nc.sync.dma_start(gb[:, 0, :], opt[nm + "_g"][li].partition_broadcast(128))
        nc.sync.dma_start(gb[:, 1, :], opt[nm + "_b"][li].partition_broadcast(128))
        return gb

    def layernorm(tiles, gb):
        for x in tiles:
            stats = sm_pool.tile([128, 3, 6], F32, tag="bnst")
            mv = sm_pool.tile([128, 2], F32, tag="bnmv")
            xg = x[:].rearrange("p (a c) -> p a c", a=3)
            for a in range(3):
                nc.vector.bn_stats(out=stats[:, a, :], in_=xg[:, a, :])
            nc.vector.bn_aggr(out=mv[:], in_=stats[:])
            rstd = sm_pool.tile([128, 1], F32, tag="rstd")
            nc.scalar.activation(out=rstd[:], in_=mv[:, 1:2],
                                 func=mybir.ActivationFunctionType.Sqrt,
                                 bias=eps_t[:], scale=1.0)
            nc.vector.reciprocal(rstd[:], rstd[:])
            nc.vector.tensor_scalar(out=x[:], in0=x[:], scalar1=mv[:, 0:1],
                                    scalar2=rstd[:],
                                    op0=mybir.AluOpType.subtract,
                                    op1=mybir.AluOpType.mult)
            if gb is not None:
                nc.vector.tensor_mul(x[:], x[:], gb[:, 0, :])
                nc.vector.tensor_add(x[:], x[:], gb[:, 1, :])

    def emit_kv_half(li, sc, h, resTc_l, kt_all, v_all):
        """KT s-half + V s-half for one head. kt[(h,m)]: [128 k, 512 s] bf16,
        v[(h,sm)]: [128 s, 768 k] bf16."""
        wk_sb, wv_sb = [], []
        for (wlist, wd) in ((wk_sb, wk_d), (wv_sb, wv_d)):
            for dc in range(DC):
                wt = w_pool.tile([128, KH], BF16, tag="w")
                nc.sync.dma_start(wt[:], wd[li, dc * 128:(dc + 1) * 128,
                                            h * KH:(h + 1) * KH])
                wlist.append(wt)
        for m in range(DC):
            pm = mm_tile()
            for dc in range(DC):
                nc.tensor.matmul(pm[:, :CW],
                                 wk_sb[dc][:, m * 128:(m + 1) * 128],
                                 resTc_l[sc][dc][:],
                                 start=(dc == 0), stop=(dc == DC - 1))
            if sc == 0:
                kt_all[(h, m)] = kt_pool.tile([128, S], BF16, tag="kt",
                                              name=f"kt{h}_{m}")
            nc.scalar.copy(out=kt_all[(h, m)][:, sc * CW:(sc + 1) * CW],
                           in_=pm[:, :CW])
        for tl in range(TPC):
            sm = sc * TPC + tl
            vt = v_pool.tile([128, D], BF16, tag="v")
            v_all[(h, sm)] = vt
            for (n0, nw) in NCH:
                pm = mm_tile()
                for dc in range(DC):
                    nc.tensor.matmul(pm[:, :nw],
                                     resTc_l[sc][dc][:, tl * 128:(tl + 1) * 128],
                                     wv_sb[dc][:, n0:n0 + nw],
                                     start=(dc == 0), stop=(dc == DC - 1))
                nc.scalar.copy(out=vt[:, n0:n0 + nw], in_=pm[:, :nw])

    def emit_qt(li, tcix, h, resTc_l):
        """QT for one head/chunk, packed 2 m's per psum bank.
        Returns 3 tiles [128, 512] bf16: tile j = m (2j, 2j+1) x 256 t."""
        wq_sb = []
        for dc in range(DC):
            wt = w_pool.tile([128, KH], BF16, tag="w")
            nc.sync.dma_start(wt[:], wq_d[li, dc * 128:(dc + 1) * 128,
                                         h * KH:(h + 1) * KH])
            wq_sb.append(wt)
        qt_sb = []
        for j in range(DC // 2):
            pm = mm_tile()
            for half in range(2):
                m = 2 * j + half
                for dc in range(DC):
                    nc.tensor.matmul(pm[:, half * CW:half * CW + CW],
                                     wq_sb[dc][:, m * 128:(m + 1) * 128],
                                     resTc_l[tcix][dc][:],
                                     start=(dc == 0), stop=(dc == DC - 1))
            ot = qt_pool.tile([128, 512], BF16, tag="qt")
            nc.vector.tensor_copy(out=ot[:], in_=pm[:])
            qt_sb.append(ot)
        return qt_sb

    def emit_scores(li, tcix, h, qt_sb, kt_all):
        """scores + exp + 1/sum diag for one head/chunk."""
        pe_list, diag_list = [], []
        for tl in range(TPC):
            pm = mm_tile()
            for kc in range(DC):
                nc.tensor.matmul(pm[:], qt_sb[kc // 2][:, (kc % 2) * CW + tl * 128:
                                                       (kc % 2) * CW + (tl + 1) * 128],
                                 kt_all[(h, kc)][:],
                                 start=(kc == 0), stop=(kc == DC - 1))
            if mask_ap is not None:
                nc.vector.tensor_add(pm[:], pm[:], mask_ap[:])
            pe = pe_pool.tile([128, S], BF16, tag="pe")
            sums = sm_pool.tile([128, 1], F32, tag="sums")
            nc.scalar.activation(out=pe[:], in_=pm[:],
                                 func=mybir.ActivationFunctionType.Exp,
                                 scale=SCALE, accum_out=sums[:])
            rec = sm_pool.tile([128, 1], F32, tag="rec")
            nc.vector.reciprocal(rec[:], sums[:])
            dg = sm_pool.tile([128, 128], BF16, tag="diag")
            nc.vector.tensor_scalar_mul(dg[:], identb[:], rec[:])
            pe_list.append(pe)
            diag_list.append(dg)
        return pe_list, diag_list

    def emit_ptco(li, tcix, h, pe_list, diag_list, v_all, acc, accb):
        """normalized P^T, ctxT, out-partial accumulate for one head/chunk."""
        # PT packed: tile j holds s-tiles (2j | 2j+1) x [2 tl x 128]
        pt_sb = []
        for j in range(TT // 2):
            pp = ps_pt.tile([128, 512], F32, tag="pt", name="ptps")
            for half in range(2):
                sm = 2 * j + half
                for tl in range(TPC):
                    nc.tensor.matmul(pp[:, half * CW + tl * 128:half * CW + (tl + 1) * 128],
                                     pe_list[tl][:, sm * 128:(sm + 1) * 128],
                                     diag_list[tl][:], start=True, stop=True)
            ps = pt_pool.tile([128, 512], BF16, tag="pts")
            nc.vector.tensor_copy(out=ps[:], in_=pp[:])
            pt_sb.append(ps)

        # ctxT packed: tile j holds km (2j | 2j+1) x 256 t
        ct_sb = []
        for j in range(DC // 2):
            pm = mm_tile()
            for half in range(2):
                km = 2 * j + half
                for sm in range(TT):
                    nc.tensor.matmul(pm[:, half * CW:half * CW + CW],
                                     v_all[(h, sm)][:, km * 128:(km + 1) * 128],
                                     pt_sb[sm // 2][:, (sm % 2) * CW:(sm % 2) * CW + CW],
                                     start=(sm == 0), stop=(sm == TT - 1))
            ot = ct_pool.tile([128, 512], BF16, tag="ct")
            nc.vector.tensor_copy(out=ot[:], in_=pm[:])
            ct_sb.append(ot)

        wo_sb = []
        for kc in range(DC):
            wt = w_pool.tile([128, D], BF16, tag="w")
            nc.sync.dma_start(wt[:], wo_d[li, h * KH + kc * 128:
                                         h * KH + (kc + 1) * 128, :])
            wo_sb.append(wt)

        for tl in range(TPC):
            for (n0, nw) in NCH:
                pm = mm_tile()
                for kc in range(DC):
                    nc.tensor.matmul(pm[:, :nw],
                                     ct_sb[kc // 2][:, (kc % 2) * CW + tl * 128:
                                                    (kc % 2) * CW + (tl + 1) * 128],
                                     wo_sb[kc][:, n0:n0 + nw],
                                     start=(kc == 0), stop=(kc == DC - 1))
                if h == 0:
                    nc.vector.tensor_copy(out=acc[tl][:, n0:n0 + nw],
                                          in_=pm[:, :nw])
                elif h < HPC - 1:
                    nc.vector.tensor_add(acc[tl][:, n0:n0 + nw],
                                         acc[tl][:, n0:n0 + nw], pm[:, :nw])
                else:
                    nc.vector.tensor_add(accb[tl][:, n0:n0 + nw],
                                         acc[tl][:, n0:n0 + nw], pm[:, :nw])

    def emit_collective(li, accb):
        arin = dram.tile([CW, D], BF16, tag="arin")
        last = li == n_layers - 1
        for tl in range(TPC):
            nc.sync.dma_start(arin[tl * 128:(tl + 1) * 128, :], accb[tl][:])
        if last:
            arout = dram.tile([128, D], BF16, tag="arout2")
            nc.gpsimd.collective_compute(
                "ReduceScatter", mybir.AluOpType.add,
                replica_groups=[[0, 1], [2, 3], [4, 5], [6, 7]],
                ins=[arin.opt()], outs=[arout.opt()])
        else:
            arout = dram.tile([CW, D], BF16, tag="arout")
            nc.gpsimd.collective_compute(
                "AllReduce", mybir.AluOpType.add,
                replica_groups=[[0, 1], [2, 3], [4, 5], [6, 7]],
                ins=[arin.opt()], outs=[arout.opt()])
        return arout

    def emit_tail_chunk(li, tcix, arout, gb1, gb2, resTc_next):
        """AR result -> LN1 -> FFN -> LN2 -> resTc_next[tcix] (or output DMA).
        For the last layer the collective was a ReduceScatter: each core owns
        128 of the 256 chunk rows; the host reassembles."""
        ntl = 1 if li == n_layers - 1 else TPC
        xcur = [accb_pool.tile([128, D], BF16, tag="accb", name=f"xcur{tl}") for tl in range(ntl)]
        for tl in range(ntl):
            nc.sync.dma_start(xcur[tl][:], arout[tl * 128:(tl + 1) * 128, :])
        layernorm(xcur, gb1)

        lnT = []
        for dc in range(DC):
            pp = ps_tp.tile([128, CW], BF16, tag="tp", name="tpps")
            for tl in range(ntl):
                nc.tensor.transpose(pp[:, tl * 128:(tl + 1) * 128],
                                    xcur[tl][:, dc * 128:(dc + 1) * 128],
                                    identb[:])
            t = pt_pool.tile([128, 512], BF16, tag="pts", name="lnT")
            nc.scalar.copy(out=t[:, :ntl * 128], in_=pp[:, :ntl * 128])
            lnT.append(t)

        ff_sb = []
        for dc in range(DC):
            wt = w_pool.tile([128, D], BF16, tag="w")
            nc.sync.dma_start(wt[:], ff_d[li, dc * 128:(dc + 1) * 128, :])
            ff_sb.append(wt)
        xmid = [xtd_pool.tile([128, D], F32, tag="xtd", name=f"xmid{tl}") for tl in range(ntl)]
        for tl in range(ntl):
            for (n0, nw) in NCH:
                pm = mm_tile()
                for dc in range(DC):
                    nc.tensor.matmul(pm[:, :nw], lnT[dc][:, tl * 128:(tl + 1) * 128],
                                     ff_sb[dc][:, n0:n0 + nw],
                                     start=(dc == 0), stop=(dc == DC - 1))
                nc.vector.tensor_copy(out=xmid[tl][:, n0:n0 + nw], in_=pm[:, :nw])

        layernorm(xmid, gb2)

        if li < n_layers - 1:
            for dc in range(DC):
                pp = ps_tp.tile([128, CW], F32, tag="tp", name="tpps")
                for tl in range(TPC):
                    nc.tensor.transpose(pp[:, tl * 128:(tl + 1) * 128],
                                        xmid[tl][:, dc * 128:(dc + 1) * 128],
                                        ident[:])
                rt = rT_pool.tile([128, CW], BF16, tag="rT")
                nc.scalar.copy(out=rt[:], in_=pp[:])
                resTc_next[tcix][dc] = rt
        else:
            nc.sync.dma_start(out_d[tcix * 128:(tcix + 1) * 128, :], xmid[0][:])

    # ---- layers --------------------------------------------------------
    # prologue: layer-0 chunk-0 KV + QT prefetch (resTc from embeddings)
    cur_kt, cur_v = {}, {}
    qt_pre = {}
    for h in range(HPC):
        emit_kv_half(0, 0, h, resTc, cur_kt, cur_v)
        if h < QT_AHEAD:
            qt_pre[h] = emit_qt(0, 0, h, resTc)

    for li in range(n_layers):
        gb1 = load_ln_gb(li, "ln1")
        gb2 = load_ln_gb(li, "ln2")

        # A: KV s-half 1 (skewed) + chunk-0 score chains
        acc0 = [xtd_pool.tile([128, D], F32, tag="xtd", name=f"acc{tl}") for tl in range(TPC)]
        accb0 = [accb_pool.tile([128, D], BF16, tag="accb", name=f"accb{tl}") for tl in range(TPC)]
        emit_kv_half(li, 1, 0, resTc, cur_kt, cur_v)
        pend = None
        for h in range(HPC):
            if h + 1 < HPC:
                emit_kv_half(li, 1, h + 1, resTc, cur_kt, cur_v)
            qt_sb = qt_pre.pop(h) if h in qt_pre else emit_qt(li, 0, h, resTc)
            sc_out = emit_scores(li, 0, h, qt_sb, cur_kt)
            if pend is not None:
                emit_ptco(li, 0, pend[0], pend[1], pend[2], cur_v, acc0, accb0)
            pend = (h, sc_out[0], sc_out[1])
        emit_ptco(li, 0, pend[0], pend[1], pend[2], cur_v, acc0, accb0)
        arout0 = emit_collective(li, accb0)

        # C: chunk-1 score chains (AR(c0) overlaps this)
        acc1 = [xtd_pool.tile([128, D], F32, tag="xtd", name=f"acc{tl}") for tl in range(TPC)]
        accb1 = [accb_pool.tile([128, D], BF16, tag="accb", name=f"accb{tl}") for tl in range(TPC)]
        pend = None
        for h in range(HPC):
            qt_sb = emit_qt(li, 1, h, resTc)
            sc_out = emit_scores(li, 1, h, qt_sb, cur_kt)
            if pend is not None:
                emit_ptco(li, 1, pend[0], pend[1], pend[2], cur_v, acc1, accb1)
            pend = (h, sc_out[0], sc_out[1])
        emit_ptco(li, 1, pend[0], pend[1], pend[2], cur_v, acc1, accb1)
        arout1 = emit_collective(li, accb1)

        # E: tail chunk 0 (AR(c0) long done)
        resTc_next = [[None] * DC for _ in range(NTC)]
        emit_tail_chunk(li, 0, arout0, gb1, gb2, resTc_next)

        # F: next layer's chunk-0 KV + QT prefetch (fills AR(c1) window)
        next_kt, next_v = {}, {}
        qt_pre = {}
        if li < n_layers - 1:
            for h in range(HPC):
                emit_kv_half(li + 1, 0, h, resTc_next, next_kt, next_v)
                if h < QT_AHEAD:
                    qt_pre[h] = emit_qt(li + 1, 0, h, resTc_next)

        # G: tail chunk 1
        emit_tail_chunk(li, 1, arout1, gb1, gb2, resTc_next)

        resTc = resTc_next
        cur_kt, cur_v = next_kt, next_v


# ------------------------------------------------------------------------
# host side
# ------------------------------------------------------------------------
_CACHED = {}
_LAST_RES = None


def _get_nc(n_layers, flag_key, flags):
    key = (n_layers, flag_key)
    if key not in _CACHED:
        _CACHED[key] = build_nc(n_layers, flags)
    return _CACHED[key]


def kernel(X, tok_w, tok_b, pos_w, pos_b, seg_w, seg_b,
           Wq, bq, Wk, bk, Wv, bv, Wo, bo,
           ln1_g, ln1_b, ffp_w, ffp_b, ln2_g, ln2_b, n_layers=L):
    global _LAST_RES
    f32 = np.float32
    X = np.asarray(X, dtype=np.int32)
    tok_w = np.asarray(tok_w, f32); pos_w = np.asarray(pos_w, f32); seg_w = np.asarray(seg_w, f32)
    Wq = np.asarray(Wq, f32); Wk = np.asarray(Wk, f32); Wv = np.asarray(Wv, f32)
    Wo = np.asarray(Wo, f32); ffp_w = np.asarray(ffp_w, f32)
    bq = np.asarray(bq, f32); bk = np.asarray(bk, f32); bv = np.asarray(bv, f32)
    bo = np.asarray(bo, f32); ffp_b = np.asarray(ffp_b, f32)
    ln1_g = np.asarray(ln1_g, f32); ln1_b = np.asarray(ln1_b, f32)
    ln2_g = np.asarray(ln2_g, f32); ln2_b = np.asarray(ln2_b, f32)
    tok_b = np.asarray(tok_b, f32); pos_b = np.asarray(pos_b, f32); seg_b = np.asarray(seg_b, f32)

    emb_bias = tok_b + pos_b + seg_b
    flags = {
        "emb_bias": bool(np.any(emb_bias)),
        "ln1": bool(np.any(ln1_g != 1) or np.any(ln1_b)),
        "ln2": bool(np.any(ln2_g != 1) or np.any(ln2_b)),
        "mask": bool(np.any(X[:, 0, :] == 0)),
    }
    assert not (np.any(bo) or np.any(ffp_b) or np.any(bq) or np.any(bk) or np.any(bv)), \
        "nonzero attention/ffn biases not implemented in this specialization"
    flag_key = tuple(sorted(flags.items()))
    nc = _get_nc(n_layers, flag_key, flags)

    tok_wb = tok_w.astype(BF)
    pos_wb = pos_w.astype(BF)
    seg_wb = seg_w.astype(BF)

    in_maps = []
    per_g = {}
    for g in range(2):
        hsl = slice(g * HPC, (g + 1) * HPC)
        per_g[g] = {
            "wq": np.ascontiguousarray(Wq[:n_layers, :, hsl, :]).reshape(n_layers, D, HK).astype(BF),
            "wk": np.ascontiguousarray(Wk[:n_layers, :, hsl, :]).reshape(n_layers, D, HK).astype(BF),
            "wv": np.ascontiguousarray(Wv[:n_layers, :, hsl, :]).reshape(n_layers, D, HK).astype(BF),
            "wo": np.ascontiguousarray(Wo[:n_layers, hsl, :, :]).reshape(n_layers, HK, D).astype(BF),
        }
    ffb = np.ascontiguousarray(ffp_w[:n_layers]).astype(BF)

    for c in range(NCORES):
        b, g = c // 2, c % 2
        m = {
            "xids": np.ascontiguousarray(X[b]),
            "tok_w": tok_wb, "pos_w": pos_wb, "seg_w": seg_wb,
            "ff": ffb,
            **per_g[g],
        }
        if flags["emb_bias"]:
            m["emb_bias"] = emb_bias
        if flags["ln1"]:
            m["ln1_g"] = np.ascontiguousarray(ln1_g[:n_layers])
            m["ln1_b"] = np.ascontiguousarray(ln1_b[:n_layers])
        if flags["ln2"]:
            m["ln2_g"] = np.ascontiguousarray(ln2_g[:n_layers])
            m["ln2_b"] = np.ascontiguousarray(ln2_b[:n_layers])
        if flags["mask"]:
            m["maskneg"] = np.where(X[b, 0, :] == 0, -1e9, 0.0).astype(f32)
        in_maps.append(m)

    res = bass_utils.run_bass_kernel_spmd(nc, in_maps, core_ids=list(range(NCORES)))
    _LAST_RES = res
    out = np.empty((B, S, D), np.float32)
    for b in range(B):
        o0 = res.results[2 * b]["out"]      # rank-0 shards: rows 0:128 / 256:384
        o1 = res.results[2 * b + 1]["out"]  # rank-1 shards: rows 128:256 / 384:512
        out[b, 0:128] = o0[0:128]
        out[b, 128:256] = o1[0:128]
        out[b, 256:384] = o0[128:256]
        out[b, 384:512] = o1[128:256]
    return out


# revision 17
# speedup vs baseline: 1.0025x; 1.0025x over previous
"""Trainium2 Bass kernel for nn_JslBERT (embedding lookup + 4-layer BERT encoder).

Sharding: 8 cores = 4 batch x 2 head-groups. Core c handles batch b=c//2 and
heads [6g, 6g+6) with g=c%2. Per layer, attention-output partials are pairwise
AllReduced in bf16; LN+FFN run redundantly on both cores of a pair.

v2 design (vs baseline):
 - weights streamed in bf16 (half DMA + half SBUF), activations f32r/bf16 mixed
 - attention phase split: KV for all heads first (per s-chunk), then the
   Q/scores/softmax/ctx/out pipeline per 256-row t-chunk
 - the per-layer AllReduce is split into two 256-row chunks so it overlaps
   with the other chunk's compute and with the next layer's KV (s-chunk 0)
 - softmax normalization folded into the P-transpose as a diag(1/sum) matmul
 - PSUM-resident accumulations, fp32 accumulate everywhere
"""
import os
import numpy as np
import ml_dtypes

import concourse.bass as bass
import concourse.bacc as bacc
import concourse.tile as tile
import concourse.bass_utils as bass_utils
from concourse import mybir
from concourse.masks import make_identity

# Model dims (hardcoded per problem spec)
B, S, L, D, H, V, PMAX = 4, 512, 4, 768, 12, 32000, 512
EPS = 1e-3
NCORES = 8
HPC = H // 2          # heads per core
KH = D                # head dim (768)
HK = HPC * KH         # 4608 flattened head dims per core
SCALE = 1.0 / float(np.sqrt(D))

F32 = mybir.dt.float32
F32R = mybir.dt.float32r
BF16 = mybir.dt.bfloat16
I32 = mybir.dt.int32

TT = S // 128         # 4 t-tiles total
DC = D // 128         # 6 d-chunks
NCH = [(0, 512), (512, 256)]  # free-dim chunks for width-768 outputs
NTC = 2               # t-chunks per sequence
TPC = TT // NTC       # 128-tiles per chunk (2)
CW = S // NTC         # chunk width (256)

BF = np.dtype(ml_dtypes.bfloat16)


def build_nc(n_layers=L, flags=None):
    """Build the Bass graph. flags: dict of which optional inputs exist."""
    flags = flags or {}
    nc = bacc.Bacc("TRN2", target_bir_lowering=False, debug=False,
                   num_devices=NCORES)

    xids_d = nc.dram_tensor("xids", [3, S], I32, kind="ExternalInput").ap()
    tokw_d = nc.dram_tensor("tok_w", [V, D], BF16, kind="ExternalInput").ap()
    posw_d = nc.dram_tensor("pos_w", [PMAX, D], BF16, kind="ExternalInput").ap()
    segw_d = nc.dram_tensor("seg_w", [2, D], BF16, kind="ExternalInput").ap()
    wq_d = nc.dram_tensor("wq", [n_layers, D, HK], BF16, kind="ExternalInput").ap()
    wk_d = nc.dram_tensor("wk", [n_layers, D, HK], BF16, kind="ExternalInput").ap()
    wv_d = nc.dram_tensor("wv", [n_layers, D, HK], BF16, kind="ExternalInput").ap()
    wo_d = nc.dram_tensor("wo", [n_layers, HK, D], BF16, kind="ExternalInput").ap()
    ff_d = nc.dram_tensor("ff", [n_layers, D, D], BF16, kind="ExternalInput").ap()
    out_d = nc.dram_tensor("out", [S, D], F32, kind="ExternalOutput").ap()

    opt = {}
    if flags.get("emb_bias"):
        opt["emb_bias"] = nc.dram_tensor("emb_bias", [D], F32, kind="ExternalInput").ap()
    for nm in ("ln1", "ln2"):
        if flags.get(nm):
            opt[nm + "_g"] = nc.dram_tensor(nm + "_g", [n_layers, D], F32, kind="ExternalInput").ap()
            opt[nm + "_b"] = nc.dram_tensor(nm + "_b", [n_layers, D], F32, kind="ExternalInput").ap()
    if flags.get("mask"):
        opt["maskneg"] = nc.dram_tensor("maskneg", [S], F32, kind="ExternalInput").ap()

    with tile.TileContext(nc) as tc:
        import contextlib
        with contextlib.ExitStack() as ctx:
            _build_body(ctx, tc, n_layers, flags, xids_d, tokw_d, posw_d, segw_d,
                        wq_d, wk_d, wv_d, wo_d, ff_d, out_d, opt)
    nc.compile()
    return nc


def _build_body(ctx, tc, n_layers, flags, xids_d, tokw_d, posw_d, segw_d,
                wq_d, wk_d, wv_d, wo_d, ff_d, out_d, opt):
    nc = tc.nc

    const = ctx.enter_context(tc.tile_pool(name="const", bufs=1))
    w_pool = ctx.enter_context(tc.tile_pool(name="wp", bufs=20))
    rT_pool = ctx.enter_context(tc.tile_pool(name="rT", bufs=24))
    kt_pool = ctx.enter_context(tc.tile_pool(name="ktp", bufs=36))
    v_pool = ctx.enter_context(tc.tile_pool(name="vp", bufs=24))
    qt_pool = ctx.enter_context(tc.tile_pool(name="qtp", bufs=21))
    pe_pool = ctx.enter_context(tc.tile_pool(name="pep", bufs=6))
    pt_pool = ctx.enter_context(tc.tile_pool(name="ptp", bufs=9))
    ct_pool = ctx.enter_context(tc.tile_pool(name="ctp", bufs=6))
    xtd_pool = ctx.enter_context(tc.tile_pool(name="xtd", bufs=6))
    accb_pool = ctx.enter_context(tc.tile_pool(name="accb", bufs=8))
    sm_pool = ctx.enter_context(tc.tile_pool(name="sm", bufs=12))
    ps_mm = ctx.enter_context(tc.tile_pool(name="psmm", bufs=6, space="PSUM"))
    ps_tp = ctx.enter_context(tc.tile_pool(name="pstp", bufs=2, space="PSUM"))
    dram = ctx.enter_context(tc.tile_pool(name="dram", bufs=2, space="DRAM"))

    ident = const.tile([128, 128], F32)
    make_identity(nc, ident[:])
    identb = const.tile([128, 128], BF16)
    make_identity(nc, identb[:])
    eps_t = const.tile([128, 1], F32)
    nc.vector.memset(eps_t[:], EPS)

    def mm_tile():
        return ps_mm.tile([128, 512], F32, tag="mm", name="mmps")

    # ---- embeddings ----------------------------------------------------
    idx = const.tile([128, 3, TT], I32)
    nc.sync.dma_start(idx[:], xids_d.rearrange("k (j p) -> p k j", p=128))

    emb_bias_ap = None
    if "emb_bias" in opt:
        eb = const.tile([128, DC], F32)
        nc.sync.dma_start(eb[:], opt["emb_bias"].rearrange("(c p) -> p c", p=128))
        emb_bias_ap = [eb[:, c:c + 1] for c in range(DC)]

    x_tiles = []
    for tm in range(TT):
        xt = accb_pool.tile([128, D], BF16, tag="accb")
        tmp = accb_pool.tile([128, D], BF16, tag="accb")
        tmp2 = accb_pool.tile([128, D], BF16, tag="accb")
        nc.gpsimd.indirect_dma_start(
            out=xt[:], out_offset=None, in_=tokw_d[:],
            in_offset=bass.IndirectOffsetOnAxis(ap=idx[:, 0, tm:tm + 1], axis=0))
        nc.gpsimd.indirect_dma_start(
            out=tmp[:], out_offset=None, in_=posw_d[:],
            in_offset=bass.IndirectOffsetOnAxis(ap=idx[:, 1, tm:tm + 1], axis=0))
        nc.vector.tensor_add(xt[:], xt[:], tmp[:])
        nc.gpsimd.indirect_dma_start(
            out=tmp2[:], out_offset=None, in_=segw_d[:],
            in_offset=bass.IndirectOffsetOnAxis(ap=idx[:, 2, tm:tm + 1], axis=0))
        nc.vector.tensor_add(xt[:], xt[:], tmp2[:])
        x_tiles.append(xt)

    # resTc[tcix][dc]: [128 d, 256 t] f32r per t-chunk
    resTc = [[None] * DC for _ in range(NTC)]
    for tcix in range(NTC):
        for dc in range(DC):
            pp = ps_tp.tile([128, CW], BF16, tag="tp", name="tpps")
            for tl in range(TPC):
                nc.tensor.transpose(pp[:, tl * 128:(tl + 1) * 128],
                                    x_tiles[tcix * TPC + tl][:, dc * 128:(dc + 1) * 128],
                                    identb[:])
            rt = rT_pool.tile([128, CW], BF16, tag="rT")
            if emb_bias_ap is not None:
                nc.vector.tensor_scalar_add(rt[:], pp[:], emb_bias_ap[dc])
            else:
                nc.vector.tensor_copy(out=rt[:], in_=pp[:])
            resTc[tcix][dc] = rt

    mask_ap = None
    if "maskneg" in opt:
        mk = const.tile([128, S], F32)
        nc.sync.dma_start(mk[:], opt["maskneg"].partition_broadcast(128))
        mask_ap = mk

    # ---- per-layer helpers --------------------------------------------
    QT_AHEAD = 6  # heads whose chunk-0 QT is prefetched at the end of the previous layer

    def load_ln_gb(li, nm):
        if nm + "_g" not in opt:
            return None
        gb = const.tile([128, 2, D], F32, tag=f"lngb{nm}{li}")
        nc.sync.dma_start(gb[:, 0, :], opt[nm + "_g"][li].partition_broadcast(128))
        nc.sync.dma_start(gb[:, 1, :], opt[nm + "_b"][li].partition_broadcast(128))
        return gb

    def layernorm(tiles, gb):
        for x in tiles:
            stats = sm_pool.tile([128, 3, 6], F32, tag="bnst")
            mv = sm_pool.tile([128, 2], F32, tag="bnmv")
            xg = x[:].rearrange("p (a c) -> p a c", a=3)
            for a in range(3):
                nc.vector.bn_stats(out=stats[:, a, :], in_=xg[:, a, :])
            nc.vector.bn_aggr(out=mv[:], in_=stats[:])
            rstd = sm_pool.tile([128, 1], F32, tag="rstd")
            nc.scalar.activation(out=rstd[:], in_=mv[:, 1:2],
                                 func=mybir.ActivationFunctionType.Sqrt,
                                 bias=eps_t[:], scale=1.0)
            nc.vector.reciprocal(rstd[:], rstd[:])
            nc.vector.tensor_scalar(out=x[:], in0=x[:], scalar1=mv[:, 0:1],
                                    scalar2=rstd[:],
                                    op0=mybir.AluOpType.subtract,
                                    op1=mybir.AluOpType.mult)
            if gb is not None:
                nc.vector.tensor_mul(x[:], x[:], gb[:, 0, :])
                nc.vector.tensor_add(x[:], x[:], gb[:, 1, :])

    def emit_kv_half(li, sc, h, resTc_l, kt_all, v_all):
        """KT s-half + V s-half for one head. kt[(h,m)]: [128 k, 512 s] bf16,
        v[(h,sm)]: [128 s, 768 k] bf16."""
        wk_sb, wv_sb = [], []
        for (wlist, wd) in ((wk_sb, wk_d), (wv_sb, wv_d)):
            for dc in range(DC):
                wt = w_pool.tile([128, KH], BF16, tag="w")
                nc.sync.dma_start(wt[:], wd[li, dc * 128:(dc + 1) * 128,
                                            h * KH:(h + 1) * KH])
                wlist.append(wt)
        for m in range(DC):
            pm = mm_tile()
            for dc in range(DC):
                nc.tensor.matmul(pm[:, :CW],
                                 wk_sb[dc][:, m * 128:(m + 1) * 128],
                                 resTc_l[sc][dc][:],
                                 start=(dc == 0), stop=(dc == DC - 1))
            if sc == 0:
                kt_all[(h, m)] = kt_pool.tile([128, S], BF16, tag="kt",
                                              name=f"kt{h}_{m}")
            nc.scalar.copy(out=kt_all[(h, m)][:, sc * CW:(sc + 1) * CW],
                           in_=pm[:, :CW])
        for tl in range(TPC):
            sm = sc * TPC + tl
            vt = v_pool.tile([128, D], BF16, tag="v")
            v_all[(h, sm)] = vt
            for (n0, nw) in NCH:
                pm = mm_tile()
                for dc in range(DC):
                    nc.tensor.matmul(pm[:, :nw],
                                     resTc_l[sc][dc][:, tl * 128:(tl + 1) * 128],
                                     wv_sb[dc][:, n0:n0 + nw],
                                     start=(dc == 0), stop=(dc == DC - 1))
                nc.scalar.copy(out=vt[:, n0:n0 + nw], in_=pm[:, :nw])

    def emit_qt(li, tcix, h, resTc_l):
        """QT for one head/chunk, packed 2 m's per psum bank.
        Returns 3 tiles [128, 512] bf16: tile j = m (2j, 2j+1) x 256 t."""
        wq_sb = []
        for dc in range(DC):
            wt = w_pool.tile([128, KH], BF16, tag="w")
            nc.sync.dma_start(wt[:], wq_d[li, dc * 128:(dc + 1) * 128,
                                         h * KH:(h + 1) * KH])
            wq_sb.append(wt)
        qt_sb = []
        for j in range(DC // 2):
            pm = mm_tile()
            for half in range(2):
                m = 2 * j + half
                for dc in range(DC):
                    nc.tensor.matmul(pm[:, half * CW:half * CW + CW],
                                     wq_sb[dc][:, m * 128:(m + 1) * 128],
                                     resTc_l[tcix][dc][:],
                                     start=(dc == 0), stop=(dc == DC - 1))
            ot = qt_pool.tile([128, 512], BF16, tag="qt")
            nc.vector.tensor_copy(out=ot[:], in_=pm[:])
            qt_sb.append(ot)
        return qt_sb

    def emit_scores(li, tcix, h, qt_sb, kt_all):
        """scores + exp + 1/sum diag for one head/chunk."""
        pe_list, diag_list = [], []
        for tl in range(TPC):
            pm = mm_tile()
            for kc in range(DC):
                nc.tensor.matmul(pm[:], qt_sb[kc // 2][:, (kc % 2) * CW + tl * 128:
                                                       (kc % 2) * CW + (tl + 1) * 128],
                                 kt_all[(h, kc)][:],
                                 start=(kc == 0), stop=(kc == DC - 1))
            if mask_ap is not None:
                nc.vector.tensor_add(pm[:], pm[:], mask_ap[:])
            pe = pe_pool.tile([128, S], BF16, tag="pe")
            sums = sm_pool.tile([128, 1], F32, tag="sums")
            nc.scalar.activation(out=pe[:], in_=pm[:],
                                 func=mybir.ActivationFunctionType.Exp,
                                 scale=SCALE, accum_out=sums[:])
            rec = sm_pool.tile([128, 1], F32, tag="rec")
            nc.vector.reciprocal(rec[:], sums[:])
            dg = sm_pool.tile([128, 128], BF16, tag="diag")
            nc.vector.tensor_scalar_mul(dg[:], identb[:], rec[:])
            pe_list.append(pe)
            diag_list.append(dg)
        return pe_list, diag_list

    def emit_ptco(li, tcix, h, pe_list, diag_list, v_all, acc, accb):
        """normalized P^T, ctxT, out-partial accumulate for one head/chunk."""
        # PT packed: tile j holds s-tiles (2j | 2j+1) x [2 tl x 128]
        pt_sb = []
        for j in range(TT // 2):
            pp = mm_tile()
            for half in range(2):
                sm = 2 * j + half
                for tl in range(TPC):
                    nc.tensor.matmul(pp[:, half * CW + tl * 128:half * CW + (tl + 1) * 128],
                                     pe_list[tl][:, sm * 128:(sm + 1) * 128],
                                     diag_list[tl][:], start=True, stop=True)
            ps = pt_pool.tile([128, 512], BF16, tag="pts")
            nc.vector.tensor_copy(out=ps[:], in_=pp[:])
            pt_sb.append(ps)

        # ctxT packed: tile j holds km (2j | 2j+1) x 256 t
        ct_sb = []
        for j in range(DC // 2):
            pm = mm_tile()
            for half in range(2):
                km = 2 * j + half
                for sm in range(TT):
                    nc.tensor.matmul(pm[:, half * CW:half * CW + CW],
                                     v_all[(h, sm)][:, km * 128:(km + 1) * 128],
                                     pt_sb[sm // 2][:, (sm % 2) * CW:(sm % 2) * CW + CW],
                                     start=(sm == 0), stop=(sm == TT - 1))
            ot = ct_pool.tile([128, 512], BF16, tag="ct")
            nc.vector.tensor_copy(out=ot[:], in_=pm[:])
            ct_sb.append(ot)

        wo_sb = []
        for kc in range(DC):
            wt = w_pool.tile([128, D], BF16, tag="w")
            nc.sync.dma_start(wt[:], wo_d[li, h * KH + kc * 128:
                                         h * KH + (kc + 1) * 128, :])
            wo_sb.append(wt)

        for tl in range(TPC):
            for (n0, nw) in NCH:
                pm = mm_tile()
                for kc in range(DC):
                    nc.tensor.matmul(pm[:, :nw],
                                     ct_sb[kc // 2][:, (kc % 2) * CW + tl * 128:
                                                    (kc % 2) * CW + (tl + 1) * 128],
                                     wo_sb[kc][:, n0:n0 + nw],
                                     start=(kc == 0), stop=(kc == DC - 1))
                if h == 0:
                    nc.vector.tensor_copy(out=acc[tl][:, n0:n0 + nw],
                                          in_=pm[:, :nw])
                elif h < HPC - 1:
                    nc.vector.tensor_add(acc[tl][:, n0:n0 + nw],
                                         acc[tl][:, n0:n0 + nw], pm[:, :nw])
                else:
                    nc.vector.tensor_add(accb[tl][:, n0:n0 + nw],
                                         acc[tl][:, n0:n0 + nw], pm[:, :nw])

    def emit_collective(li, accb):
        arin = dram.tile([CW, D], BF16, tag="arin")
        last = li == n_layers - 1
        for tl in range(TPC):
            nc.sync.dma_start(arin[tl * 128:(tl + 1) * 128, :], accb[tl][:])
        if last:
            arout = dram.tile([128, D], BF16, tag="arout2")
            nc.gpsimd.collective_compute(
                "ReduceScatter", mybir.AluOpType.add,
                replica_groups=[[0, 1], [2, 3], [4, 5], [6, 7]],
                ins=[arin.opt()], outs=[arout.opt()])
        else:
            arout = dram.tile([CW, D], BF16, tag="arout")
            nc.gpsimd.collective_compute(
                "AllReduce", mybir.AluOpType.add,
                replica_groups=[[0, 1], [2, 3], [4, 5], [6, 7]],
                ins=[arin.opt()], outs=[arout.opt()])
        return arout

    def emit_tail_chunk(li, tcix, arout, gb1, gb2, resTc_next):
        """AR result -> LN1 -> FFN -> LN2 -> resTc_next[tcix] (or output DMA).
        For the last layer the collective was a ReduceScatter: each core owns
        128 of the 256 chunk rows; the host reassembles."""
        ntl = 1 if li == n_layers - 1 else TPC
        xcur = [accb_pool.tile([128, D], BF16, tag="accb", name=f"xcur{tl}") for tl in range(ntl)]
        for tl in range(ntl):
            nc.sync.dma_start(xcur[tl][:], arout[tl * 128:(tl + 1) * 128, :])
        layernorm(xcur, gb1)

        lnT = []
        for dc in range(DC):
            pp = ps_tp.tile([128, CW], BF16, tag="tp", name="tpps")
            for tl in range(ntl):
                nc.tensor.transpose(pp[:, tl * 128:(tl + 1) * 128],
                                    xcur[tl][:, dc * 128:(dc + 1) * 128],
                                    identb[:])
            t = pt_pool.tile([128, 512], BF16, tag="pts", name="lnT")
            nc.scalar.copy(out=t[:, :ntl * 128], in_=pp[:, :ntl * 128])
            lnT.append(t)

        ff_sb = []
        for dc in range(DC):
            wt = w_pool.tile([128, D], BF16, tag="w")
            nc.sync.dma_start(wt[:], ff_d[li, dc * 128:(dc + 1) * 128, :])
            ff_sb.append(wt)
        xmid = [xtd_pool.tile([128, D], F32, tag="xtd", name=f"xmid{tl}") for tl in range(ntl)]
        for tl in range(ntl):
            for (n0, nw) in NCH:
                pm = mm_tile()
                for dc in range(DC):
                    nc.tensor.matmul(pm[:, :nw], lnT[dc][:, tl * 128:(tl + 1) * 128],
                                     ff_sb[dc][:, n0:n0 + nw],
                                     start=(dc == 0), stop=(dc == DC - 1))
                nc.vector.tensor_copy(out=xmid[tl][:, n0:n0 + nw], in_=pm[:, :nw])

        layernorm(xmid, gb2)

        if li < n_layers - 1:
            for dc in range(DC):
                pp = ps_tp.tile([128, CW], F32, tag="tp", name="tpps")
                for tl in range(TPC):
                    nc.tensor.transpose(pp[:, tl * 128:(tl + 1) * 128],
                                        xmid[tl][:, dc * 128:(dc + 1) * 128],
                                        ident[:])
                rt = rT_pool.tile([128, CW], BF16, tag="rT")
                nc.scalar.copy(out=rt[:], in_=pp[:])
                resTc_next[tcix][dc] = rt
        else:
            nc.sync.dma_start(out_d[tcix * 128:(tcix + 1) * 128, :], xmid[0][:])

    # ---- layers --------------------------------------------------------
    # prologue: layer-0 chunk-0 KV + QT prefetch (resTc from embeddings)
    cur_kt, cur_v = {}, {}
    qt_pre = {}
    for h in range(HPC):
        emit_kv_half(0, 0, h, resTc, cur_kt, cur_v)
        if h < QT_AHEAD:
            qt_pre[h] = emit_qt(0, 0, h, resTc)

    for li in range(n_layers):
        gb1 = load_ln_gb(li, "ln1")
        gb2 = load_ln_gb(li, "ln2")

        # A: KV s-half 1 (skewed) + chunk-0 score chains
        acc0 = [xtd_pool.tile([128, D], F32, tag="xtd", name=f"acc{tl}") for tl in range(TPC)]
        accb0 = [accb_pool.tile([128, D], BF16, tag="accb", name=f"accb{tl}") for tl in range(TPC)]
        emit_kv_half(li, 1, 0, resTc, cur_kt, cur_v)
        pend = None
        for h in range(HPC):
            if h + 1 < HPC:
                emit_kv_half(li, 1, h + 1, resTc, cur_kt, cur_v)
            qt_sb = qt_pre.pop(h) if h in qt_pre else emit_qt(li, 0, h, resTc)
            sc_out = emit_scores(li, 0, h, qt_sb, cur_kt)
            if pend is not None:
                emit_ptco(li, 0, pend[0], pend[1], pend[2], cur_v, acc0, accb0)
            pend = (h, sc_out[0], sc_out[1])
        emit_ptco(li, 0, pend[0], pend[1], pend[2], cur_v, acc0, accb0)
        arout0 = emit_collective(li, accb0)

        # C: chunk-1 score chains (AR(c0) overlaps this)
        acc1 = [xtd_pool.tile([128, D], F32, tag="xtd", name=f"acc{tl}") for tl in range(TPC)]
        accb1 = [accb_pool.tile([128, D], BF16, tag="accb", name=f"accb{tl}") for tl in range(TPC)]
        pend = None
        for h in range(HPC):
            qt_sb = emit_qt(li, 1, h, resTc)
            sc_out = emit_scores(li, 1, h, qt_sb, cur_kt)
            if pend is not None:
                emit_ptco(li, 1, pend[0], pend[1], pend[2], cur_v, acc1, accb1)
            pend = (h, sc_out[0], sc_out[1])
        emit_ptco(li, 1, pend[0], pend[1], pend[2], cur_v, acc1, accb1)
        arout1 = emit_collective(li, accb1)

        # E: tail chunk 0 (AR(c0) long done)
        resTc_next = [[None] * DC for _ in range(NTC)]
        emit_tail_chunk(li, 0, arout0, gb1, gb2, resTc_next)

        # F: next layer's chunk-0 KV + QT prefetch (fills AR(c1) window)
        next_kt, next_v = {}, {}
        qt_pre = {}
        if li < n_layers - 1:
            for h in range(HPC):
                emit_kv_half(li + 1, 0, h, resTc_next, next_kt, next_v)
                if h < QT_AHEAD:
                    qt_pre[h] = emit_qt(li + 1, 0, h, resTc_next)

        # G: tail chunk 1
        emit_tail_chunk(li, 1, arout1, gb1, gb2, resTc_next)

        resTc = resTc_next
        cur_kt, cur_v = next_kt, next_v


# ------------------------------------------------------------------------
# host side
# ------------------------------------------------------------------------
_CACHED = {}
_LAST_RES = None


def _get_nc(n_layers, flag_key, flags):
    key = (n_layers, flag_key)
    if key not in _CACHED:
        _CACHED[key] = build_nc(n_layers, flags)
    return _CACHED[key]


def kernel(X, tok_w, tok_b, pos_w, pos_b, seg_w, seg_b,
           Wq, bq, Wk, bk, Wv, bv, Wo, bo,
           ln1_g, ln1_b, ffp_w, ffp_b, ln2_g, ln2_b, n_layers=L):
    global _LAST_RES
    f32 = np.float32
    X = np.asarray(X, dtype=np.int32)
    tok_w = np.asarray(tok_w, f32); pos_w = np.asarray(pos_w, f32); seg_w = np.asarray(seg_w, f32)
    Wq = np.asarray(Wq, f32); Wk = np.asarray(Wk, f32); Wv = np.asarray(Wv, f32)
    Wo = np.asarray(Wo, f32); ffp_w = np.asarray(ffp_w, f32)
    bq = np.asarray(bq, f32); bk = np.asarray(bk, f32); bv = np.asarray(bv, f32)
    bo = np.asarray(bo, f32); ffp_b = np.asarray(ffp_b, f32)
    ln1_g = np.asarray(ln1_g, f32); ln1_b = np.asarray(ln1_b, f32)
    ln2_g = np.asarray(ln2_g, f32); ln2_b = np.asarray(ln2_b, f32)
    tok_b = np.asarray(tok_b, f32); pos_b = np.asarray(pos_b, f32); seg_b = np.asarray(seg_b, f32)

    emb_bias = tok_b + pos_b + seg_b
    flags = {
        "emb_bias": bool(np.any(emb_bias)),
        "ln1": bool(np.any(ln1_g != 1) or np.any(ln1_b)),
        "ln2": bool(np.any(ln2_g != 1) or np.any(ln2_b)),
        "mask": bool(np.any(X[:, 0, :] == 0)),
    }
    assert not (np.any(bo) or np.any(ffp_b) or np.any(bq) or np.any(bk) or np.any(bv)), \
        "nonzero attention/ffn biases not implemented in this specialization"
    flag_key = tuple(sorted(flags.items()))
    nc = _get_nc(n_layers, flag_key, flags)

    tok_wb = tok_w.astype(BF)
    pos_wb = pos_w.astype(BF)
    seg_wb = seg_w.astype(BF)

    in_maps = []
    per_g = {}
    for g in range(2):
        hsl = slice(g * HPC, (g + 1) * HPC)
        per_g[g] = {
            "wq": np.ascontiguousarray(Wq[:n_layers, :, hsl, :]).reshape(n_layers, D, HK).astype(BF),
            "wk": np.ascontiguousarray(Wk[:n_layers, :, hsl, :]).reshape(n_layers, D, HK).astype(BF),
            "wv": np.ascontiguousarray(Wv[:n_layers, :, hsl, :]).reshape(n_layers, D, HK).astype(BF),
            "wo": np.ascontiguousarray(Wo[:n_layers, hsl, :, :]).reshape(n_layers, HK, D).astype(BF),
        }
    ffb = np.ascontiguousarray(ffp_w[:n_layers]).astype(BF)

    for c in range(NCORES):
        b, g = c // 2, c % 2
        m = {
            "xids": np.ascontiguousarray(X[b]),
            "tok_w": tok_wb, "pos_w": pos_wb, "seg_w": seg_wb,
            "ff": ffb,
            **per_g[g],
        }
        if flags["emb_bias"]:
            m["emb_bias"] = emb_bias
        if flags["ln1"]:
            m["ln1_g"] = np.ascontiguousarray(ln1_g[:n_layers])
            m["ln1_b"] = np.ascontiguousarray(ln1_b[:n_layers])
        if flags["ln2"]:
            m["ln2_g"] = np.ascontiguousarray(ln2_g[:n_layers])
            m["ln2_b"] = np.ascontiguousarray(ln2_b[:n_layers])
        if flags["mask"]:
            m["maskneg"] = np.where(X[b, 0, :] == 0, -1e9, 0.0).astype(f32)
        in_maps.append(m)

    res = bass_utils.run_bass_kernel_spmd(nc, in_maps, core_ids=list(range(NCORES)))
    _LAST_RES = res
    out = np.empty((B, S, D), np.float32)
    for b in range(B):
        o0 = res.results[2 * b]["out"]      # rank-0 shards: rows 0:128 / 256:384
        o1 = res.results[2 * b + 1]["out"]  # rank-1 shards: rows 128:256 / 384:512
        out[b, 0:128] = o0[0:128]
        out[b, 128:256] = o1[0:128]
        out[b, 256:384] = o0[128:256]
        out[b, 384:512] = o1[128:256]
    return out
